# revision 2
# baseline (speedup 1.0000x reference)
"""Bass/Trainium2 kernel for AttentionMessagePassing (gnn_message_passing).

Math per batch b (N=128 nodes, F=Fe=64):
  att|conv[i,j,l] = A[i,j]*(E[i,j]@We + H[j]@Wj + H[i]@Wi)[l] + bias[l]
  out[j,l] = sum_i sigmoid(att[..., 0:64]) * relu(conv[..., 64:128])

Sharding: data-parallel over batch, B=8 -> one batch element per NeuronCore.

Per-core device program ([j partitions, l free] psum tiles, 16 waves of 8 i's):
  - warm-up matmuls on a scratch tile ramp the PE p-state during DMA lead-in
  - ETH [128, 16384] bf16 host-pre-transposed (E*A | Hj*A stacked on the
    contraction dim), streamed as wave-aligned 1024-col chunk DMAs; consts
    ride a second (Activation) DGE queue to land concurrently
  - per i: K=2 addend matmul (A row; ones)x(HiW row; bias) opening the psum
    group, then K=128 matmul ETH-block x [We|Wj] closing it -> att|conv
  - per wave: sigmoid on ACT; gate product relu*sig on DVE
    (scalar_tensor_tensor from psum) or, for ACT-assisted waves, ACT
    relu-copies conv to SBUF and GPSIMD multiplies (GPSIMD cannot read PSUM);
    bf16 add-tree over the 8 i's at DVE 2x; running bf16 acc, f32 final fold
"""

import numpy as np
import ml_dtypes

B, N, FN, FE = 8, 128, 64, 64
NT = N * N
L2 = 2 * FN

IPW = 8
NWAVE = N // IPW          # 16
PW = IPW * L2             # 1024
NWARM = 5                 # warm-up matmuls
ACT_ASSIST = frozenset((1, 2, 4, 6, 8, 10, 12, 14))

_CACHE = {}


def _build_program():
    import concourse.mybir as mybir
    from concourse import bacc
    from concourse.tile import TileContext

    nc = bacc.Bacc("TRN2", target_bir_lowering=False, debug=False)

    bf16 = mybir.dt.bfloat16
    f32 = mybir.dt.float32

    eth_d = nc.dram_tensor("ETH", [L2, NT], bf16, kind="ExternalInput").ap()
    ww_d = nc.dram_tensor("WW", [L2, L2], bf16, kind="ExternalInput").ap()
    ah_d = nc.dram_tensor("AH", [2, 2 * NT], bf16, kind="ExternalInput").ap()
    out_d = nc.dram_tensor("OUT", [N, FN], f32, kind="ExternalOutput").ap()

    with TileContext(nc) as tc:
        with tc.tile_pool(name="const", bufs=1) as cpool, \
             tc.tile_pool(name="big", bufs=1) as bpool, \
             tc.tile_pool(name="psum", bufs=4, space="PSUM") as ppool, \
             tc.tile_pool(name="work", bufs=3) as tpool:

            # PE warm-up: ramp the p-state while DMAs are in flight.
            # The warm tile shares the "ps" ring slot; it has no readers so
            # the slot recycles for free.
            scratch = cpool.tile([L2, 512], bf16)
            nc.vector.memset(scratch, 0.0)

            ah_sb = cpool.tile([2, 2 * NT], bf16)
            nc.sync.dma_start(out=ah_sb, in_=ah_d)
            a2_sb = ah_sb[:, 0:NT]
            hb2_sb = ah_sb[:, NT:2 * NT]

            # ww rides the Activation DGE queue, landing concurrently
            # with the SP-queue consts; chunk 0 is split for earlier start
            ww_sb = cpool.tile([L2, L2], bf16)
            nc.scalar.dma_start(out=ww_sb, in_=ww_d)
            eth = bpool.tile([L2, NT], bf16)
            nc.sync.dma_start(out=eth[:, 0:PW // 2], in_=eth_d[:, 0:PW // 2])
            nc.sync.dma_start(out=eth[:, PW // 2:PW],
                              in_=eth_d[:, PW // 2:PW])

            # trigger the sigmoid act-table load before real work arrives
            # (after the DMA issues above — it would block the ACT sequencer)
            dscratch = cpool.tile([L2, 2], bf16)
            nc.scalar.activation(
                out=dscratch, in_=scratch[:, 0:2],
                func=mybir.ActivationFunctionType.Sigmoid)
            wps = ppool.tile([N, PW], f32, tag="ps")
            for _ in range(NWARM):
                nc.tensor.matmul(out=wps[:, 0:512], lhsT=scratch[:, 0:L2],
                                 rhs=scratch, start=True, stop=True)

            for c in range(1, NWAVE):
                nc.sync.dma_start(
                    out=eth[:, c * PW:(c + 1) * PW],
                    in_=eth_d[:, c * PW:(c + 1) * PW])

            accs = [None]   # running f32 accumulator

            for w in range(NWAVE):
                ps = ppool.tile([N, PW], f32, tag="ps")
                ps3 = ps.rearrange("p (t l) -> p t l", l=L2)
                for t in range(IPW):
                    i = w * IPW + t
                    nc.tensor.matmul(
                        out=ps[:, t * L2:(t + 1) * L2],
                        lhsT=a2_sb[:, i * N:(i + 1) * N],
                        rhs=hb2_sb[:, i * N:(i + 1) * N],
                        start=True, stop=False)
                    nc.tensor.matmul(
                        out=ps[:, t * L2:(t + 1) * L2],
                        lhsT=eth[:, i * N:(i + 1) * N],
                        rhs=ww_sb,
                        start=False, stop=True)
                ssl = tpool.tile([N, IPW, FN], bf16, tag=f"sig{w % 3}")
                nc.scalar.activation(
                    out=ssl, in_=ps3[:, :, 0:FN],
                    func=mybir.ActivationFunctionType.Sigmoid)
                # gsl in packed [j, (t, l)] layout: DVE tree adds run at 2x
                gsl = tpool.tile([N, IPW, FN], bf16, tag=f"gat{w % 3}")
                if w in ACT_ASSIST:
                    # ACT copies conv out of psum (table-free); GPSIMD does
                    # relu+mult from SBUF (its only legal data path)
                    rsl = tpool.tile([N, IPW, FN], bf16, tag=f"rel{w % 3}")
                    nc.scalar.activation(
                        out=rsl, in_=ps3[:, :, FN:L2],
                        func=mybir.ActivationFunctionType.Relu)
                    nc.gpsimd.tensor_tensor(
                        out=gsl, in0=rsl, in1=ssl,
                        op=mybir.AluOpType.mult)
                else:
                    nc.vector.scalar_tensor_tensor(
                        out=gsl, in0=ps3[:, :, FN:L2], scalar=0.0, in1=ssl,
                        op0=mybir.AluOpType.max, op1=mybir.AluOpType.mult)
                # bf16 tree over the 8 i's; running bf16 acc, f32 final
                last = w == NWAVE - 1
                with nc.allow_low_precision("bf16 partials, f32 final fold"):
                    g4 = tpool.tile([N, 4, FN], bf16, tag=f"g4_{w % 3}")
                    nc.vector.tensor_add(
                        out=g4, in0=gsl[:, 0:4, :], in1=gsl[:, 4:8, :])
                    g2 = tpool.tile([N, 2, FN], bf16, tag=f"g2_{w % 3}")
                    nc.vector.tensor_add(
                        out=g2, in0=g4[:, 0:2, :], in1=g4[:, 2:4, :])
                    red = tpool.tile([N, FN], bf16, tag=f"red{w % 3}")
                    nc.vector.tensor_add(
                        out=red, in0=g2[:, 0, :], in1=g2[:, 1, :])
                    if accs[0] is None:
                        accs[0] = red
                    else:
                        dt = f32 if last else bf16
                        nxt = tpool.tile([N, FN], dt, tag=f"acc{w % 2}")
                        nc.vector.tensor_add(out=nxt, in0=accs[0], in1=red)
                        accs[0] = nxt

            nc.sync.dma_start(out=out_d, in_=accs[0])

    nc.compile()
    return nc


def _host_prep(H, A, E, W_att, W_nei, bias_att, bias_nei):
    bf = ml_dtypes.bfloat16
    f32 = np.float32
    H, A, E = H.astype(f32), A.astype(f32), E.astype(f32)
    Wi = np.hstack([W_att[:FN], W_nei[:FN]]).astype(f32)          # [64,128]
    Wj = np.hstack([W_att[FN:2 * FN], W_nei[FN:2 * FN]]).astype(f32)
    We = np.hstack([W_att[2 * FN:], W_nei[2 * FN:]]).astype(f32)
    WW = np.ascontiguousarray(np.vstack([We, Wj]).astype(bf))      # [128,128]
    bias_both = np.concatenate([bias_att, bias_nei]).astype(f32)   # [128]

    Acol = A[..., None]
    EA = E * Acol
    HjA = Acol * H[:, None, :, :]
    X = np.concatenate([EA, HjA], axis=3)                          # [B,i,j,128]
    ETH = np.ascontiguousarray(
        X.transpose(0, 3, 1, 2).reshape(B, L2, NT).astype(bf))
    HiW = H @ Wi                                                   # [B,128,128]
    ones_row = np.ones(NT, f32)
    bias_tiled = np.tile(bias_both, N)

    in_maps = []
    for b in range(B):
        AH = np.concatenate([
            np.stack([A[b].reshape(NT), ones_row]),
            np.stack([HiW[b].reshape(NT), bias_tiled])], axis=1)
        in_maps.append({
            "ETH": ETH[b],
            "WW": WW,
            "AH": np.ascontiguousarray(AH.astype(bf)),
        })
    return in_maps


def kernel(H, A, E, W_att, W_nei, bias_att, bias_nei, N=None, **kw):
    from concourse import bass_utils

    if "nc" not in _CACHE:
        _CACHE["nc"] = _build_program()
    nc = _CACHE["nc"]
    in_maps = _host_prep(H, A, E, W_att, W_nei, bias_att, bias_nei)
    res = bass_utils.run_bass_kernel_spmd(nc, in_maps, core_ids=list(range(B)))
    out = np.stack([res.results[b]["OUT"] for b in range(B)]).astype(np.float32)
    _CACHE["last_results"] = res
    return out


# revision 13
# speedup vs baseline: 1.0036x; 1.0036x over previous
"""Bass/Trainium2 kernel for AttentionMessagePassing (gnn_message_passing).

Math per batch b (N=128 nodes, F=Fe=64):
  att|conv[i,j,l] = A[i,j]*(E[i,j]@We + H[j]@Wj + H[i]@Wi)[l] + bias[l]
  out[j,l] = sum_i sigmoid(att[..., 0:64]) * relu(conv[..., 64:128])

Sharding: data-parallel over batch, B=8 -> one batch element per NeuronCore.

Per-core device program ([j partitions, l free] psum tiles, 16 waves of 8 i's):
  - warm-up matmuls on a scratch tile ramp the PE p-state during DMA lead-in
  - ETH [128, 16384] bf16 host-pre-transposed (E*A | Hj*A stacked on the
    contraction dim), streamed as wave-aligned 1024-col chunk DMAs; consts
    ride a second (Activation) DGE queue to land concurrently
  - per i: K=2 addend matmul (A row; ones)x(HiW row; bias) opening the psum
    group, then K=128 matmul ETH-block x [We|Wj] closing it -> att|conv
  - per wave: sigmoid on ACT; gate product relu*sig on DVE
    (scalar_tensor_tensor from psum) or, for ACT-assisted waves, ACT
    relu-copies conv to SBUF and GPSIMD multiplies (GPSIMD cannot read PSUM);
    bf16 add-tree over the 8 i's at DVE 2x; running bf16 acc, f32 final fold
"""

import numpy as np
import ml_dtypes

B, N, FN, FE = 8, 128, 64, 64
NT = N * N
L2 = 2 * FN

IPW = 8
NWAVE = N // IPW          # 16
PW = IPW * L2             # 1024
NWARM = 5                 # warm-up matmuls
ACT_ASSIST = frozenset((0, 2, 4, 6, 8, 10, 12, 14))

_CACHE = {}


def _build_program():
    import concourse.mybir as mybir
    from concourse import bacc
    from concourse.tile import TileContext

    nc = bacc.Bacc("TRN2", target_bir_lowering=False, debug=False)

    bf16 = mybir.dt.bfloat16
    f32 = mybir.dt.float32

    eth_d = nc.dram_tensor("ETH", [L2, NT], bf16, kind="ExternalInput").ap()
    ww_d = nc.dram_tensor("WW", [L2, L2], bf16, kind="ExternalInput").ap()
    ah_d = nc.dram_tensor("AH", [2, 2 * NT], bf16, kind="ExternalInput").ap()
    out_d = nc.dram_tensor("OUT", [N, FN], f32, kind="ExternalOutput").ap()

    with TileContext(nc) as tc:
        with tc.tile_pool(name="const", bufs=1) as cpool, \
             tc.tile_pool(name="big", bufs=1) as bpool, \
             tc.tile_pool(name="psum", bufs=4, space="PSUM") as ppool, \
             tc.tile_pool(name="work", bufs=3) as tpool:

            # PE warm-up: ramp the p-state while DMAs are in flight.
            # The warm tile shares the "ps" ring slot; it has no readers so
            # the slot recycles for free.
            scratch = cpool.tile([L2, 512], bf16)
            nc.vector.memset(scratch, 0.0)

            ah_sb = cpool.tile([2, 2 * NT], bf16)
            nc.sync.dma_start(out=ah_sb, in_=ah_d)
            a2_sb = ah_sb[:, 0:NT]
            hb2_sb = ah_sb[:, NT:2 * NT]

            # ww rides the Activation DGE queue, landing concurrently
            # with the SP-queue consts; chunk 0 is split for earlier start
            ww_sb = cpool.tile([L2, L2], bf16)
            nc.scalar.dma_start(out=ww_sb, in_=ww_d)
            eth = bpool.tile([L2, NT], bf16)
            nc.sync.dma_start(out=eth[:, 0:PW // 2], in_=eth_d[:, 0:PW // 2])
            nc.sync.dma_start(out=eth[:, PW // 2:PW],
                              in_=eth_d[:, PW // 2:PW])

            # trigger the sigmoid act-table load before real work arrives
            # (after the DMA issues above — it would block the ACT sequencer)
            dscratch = cpool.tile([L2, 2], bf16)
            nc.scalar.activation(
                out=dscratch, in_=scratch[:, 0:2],
                func=mybir.ActivationFunctionType.Sigmoid)
            wps = ppool.tile([N, PW], f32, tag="ps")
            for _ in range(NWARM):
                nc.tensor.matmul(out=wps[:, 0:512], lhsT=scratch[:, 0:L2],
                                 rhs=scratch, start=True, stop=True)

            for c in range(1, NWAVE):
                nc.sync.dma_start(
                    out=eth[:, c * PW:(c + 1) * PW],
                    in_=eth_d[:, c * PW:(c + 1) * PW])

            accs = [None]   # running f32 accumulator

            for w in range(NWAVE):
                ps = ppool.tile([N, PW], f32, tag="ps")
                ps3 = ps.rearrange("p (t l) -> p t l", l=L2)
                for t in range(IPW):
                    i = w * IPW + t
                    nc.tensor.matmul(
                        out=ps[:, t * L2:(t + 1) * L2],
                        lhsT=a2_sb[:, i * N:(i + 1) * N],
                        rhs=hb2_sb[:, i * N:(i + 1) * N],
                        start=True, stop=False)
                    nc.tensor.matmul(
                        out=ps[:, t * L2:(t + 1) * L2],
                        lhsT=eth[:, i * N:(i + 1) * N],
                        rhs=ww_sb,
                        start=False, stop=True)
                ssl = tpool.tile([N, IPW, FN], bf16, tag=f"sig{w % 3}")
                nc.scalar.activation(
                    out=ssl, in_=ps3[:, :, 0:FN],
                    func=mybir.ActivationFunctionType.Sigmoid)
                # gsl in packed [j, (t, l)] layout: DVE tree adds run at 2x
                gsl = tpool.tile([N, IPW, FN], bf16, tag=f"gat{w % 3}")
                if w in ACT_ASSIST:
                    # ACT copies conv out of psum (table-free); GPSIMD does
                    # relu+mult from SBUF (its only legal data path)
                    rsl = tpool.tile([N, IPW, FN], bf16, tag=f"rel{w % 3}")
                    nc.scalar.activation(
                        out=rsl, in_=ps3[:, :, FN:L2],
                        func=mybir.ActivationFunctionType.Relu)
                    nc.gpsimd.tensor_tensor(
                        out=gsl, in0=rsl, in1=ssl,
                        op=mybir.AluOpType.mult)
                else:
                    nc.vector.scalar_tensor_tensor(
                        out=gsl, in0=ps3[:, :, FN:L2], scalar=0.0, in1=ssl,
                        op0=mybir.AluOpType.max, op1=mybir.AluOpType.mult)
                # bf16 tree over the 8 i's; running bf16 acc, f32 final
                last = w == NWAVE - 1
                with nc.allow_low_precision("bf16 partials, f32 final fold"):
                    g4 = tpool.tile([N, 4, FN], bf16, tag=f"g4_{w % 3}")
                    nc.vector.tensor_add(
                        out=g4, in0=gsl[:, 0:4, :], in1=gsl[:, 4:8, :])
                    g2 = tpool.tile([N, 2, FN], bf16, tag=f"g2_{w % 3}")
                    nc.vector.tensor_add(
                        out=g2, in0=g4[:, 0:2, :], in1=g4[:, 2:4, :])
                    red = tpool.tile([N, FN], bf16, tag=f"red{w % 3}")
                    nc.vector.tensor_add(
                        out=red, in0=g2[:, 0, :], in1=g2[:, 1, :])
                    if accs[0] is None:
                        accs[0] = red
                    else:
                        dt = f32 if last else bf16
                        nxt = tpool.tile([N, FN], dt, tag=f"acc{w % 2}")
                        nc.vector.tensor_add(out=nxt, in0=accs[0], in1=red)
                        accs[0] = nxt

            nc.sync.dma_start(out=out_d, in_=accs[0])

    nc.compile()
    return nc


def _host_prep(H, A, E, W_att, W_nei, bias_att, bias_nei):
    bf = ml_dtypes.bfloat16
    f32 = np.float32
    H, A, E = H.astype(f32), A.astype(f32), E.astype(f32)
    Wi = np.hstack([W_att[:FN], W_nei[:FN]]).astype(f32)          # [64,128]
    Wj = np.hstack([W_att[FN:2 * FN], W_nei[FN:2 * FN]]).astype(f32)
    We = np.hstack([W_att[2 * FN:], W_nei[2 * FN:]]).astype(f32)
    WW = np.ascontiguousarray(np.vstack([We, Wj]).astype(bf))      # [128,128]
    bias_both = np.concatenate([bias_att, bias_nei]).astype(f32)   # [128]

    Acol = A[..., None]
    EA = E * Acol
    HjA = Acol * H[:, None, :, :]
    X = np.concatenate([EA, HjA], axis=3)                          # [B,i,j,128]
    ETH = np.ascontiguousarray(
        X.transpose(0, 3, 1, 2).reshape(B, L2, NT).astype(bf))
    HiW = H @ Wi                                                   # [B,128,128]
    ones_row = np.ones(NT, f32)
    bias_tiled = np.tile(bias_both, N)

    in_maps = []
    for b in range(B):
        AH = np.concatenate([
            np.stack([A[b].reshape(NT), ones_row]),
            np.stack([HiW[b].reshape(NT), bias_tiled])], axis=1)
        in_maps.append({
            "ETH": ETH[b],
            "WW": WW,
            "AH": np.ascontiguousarray(AH.astype(bf)),
        })
    return in_maps


def kernel(H, A, E, W_att, W_nei, bias_att, bias_nei, N=None, **kw):
    from concourse import bass_utils

    if "nc" not in _CACHE:
        _CACHE["nc"] = _build_program()
    nc = _CACHE["nc"]
    in_maps = _host_prep(H, A, E, W_att, W_nei, bias_att, bias_nei)
    res = bass_utils.run_bass_kernel_spmd(nc, in_maps, core_ids=list(range(B)))
    out = np.stack([res.results[b]["OUT"] for b in range(B)]).astype(np.float32)
    _CACHE["last_results"] = res
    return out
